# revision 9
# baseline (speedup 1.0000x reference)
"""Trainium2 Bass kernel for the AdvancedFuser problem.

Computes, for each batch row b:
    w        = softmax(retrieved_weights)                       # (5,), host
    weighted = sum_k w[k] * retrieved[b, k, :]                  # (512,)
    gate     = sigmoid(q[b] . g1 + weighted . g2 + gate_b)      # scalar
    out[b]   = gate * q[b] + (1 - gate) * weighted

Sharding: pure data parallel over 8 NeuronCores (8192 rows each). The tiny
params (softmax weights, gate vector) are folded into immediates / small
replicated constant tensors on the host.

The kernel is memory bound: 112 MiB HBM traffic per core (96 in, 16 out)
against the ~358 GB/s per-core HBM share (716 GB/s per stack, 2 cores per
stack) -> ~328 us floor.

Device program (row layout, batch rows on SBUF partitions):
  - The host concatenates retrieved (2560 f32) and q (512 f32) per row into
    one [rows, 3072] tensor so loads are a single sequential HBM stream with
    12 KiB descriptors (vs separate 10 KiB r + 2 KiB q streams).
  - 2-tile supertiles (3 MiB loads), bufs=6 on the load pool: the HWDGE
    FIFO head-of-line wait quantum is halved vs 4-tile supertiles while
    keeping ~18 MiB of prefetch runway, so the SDMA engines stay fed.
    1-tile supertiles at both ends for fast pipeline fill/drain.
  - mode "hy" (default) alternates the weighted-sum engine per 128-row
    tile (even: TensorE 5 accumulating diag(w_k) matmuls in PSUM; odd: DVE
    chain of 4 fused scalar_tensor_tensor with the w_k ratio trick) so
    neither engine alone paces the DMA stream.
  - the two 512-wide per-row dots as fused mul+accumulate on DVE;
    gate = Sigmoid(s2 + s1) on ScalarE; qg = gate*q, wN = gateN*weighted
    via per-partition activation scales on ScalarE; out = qg + wN on GPSIMD.
  - loads ride the Sync HWDGE ring (a pure DMA-issue queue), stores +
    constants the ScalarE HWDGE ring.
"""

import os
import sys

import numpy as np

N_CORES = 8
BATCH = 65536
D = 512
K = 5
RB = K * D  # 2560 floats of retrieved per row
CB = RB + D  # 3072 floats per combined row: [r0..r4 | q]
ROWS = BATCH // N_CORES  # 8192
N_TILES = ROWS // 128  # 64

# Filled by the most recent kernel() call when tracing is enabled.
LAST_EXEC_NS = None
LAST_RESULTS = None

_PROGRAM_CACHE = {}


def _install_ntff_hook_shim():
    """Provide antenv.axon_hooks (missing in this image) so that
    run_bass_kernel_spmd(trace=True) can capture NTFF profiles through the
    axon PJRT .so. Mirrors trn_agent_boot.trn_boot._ntff_profile_via_ctypes."""
    try:
        from antenv.axon_hooks import get_axon_ntff_profile_hook  # noqa: F401

        return
    except ImportError:
        pass
    import contextlib
    import ctypes
    import types

    so_path = "/opt/axon/libaxon_pjrt.so"
    hook = None
    try:
        lib = ctypes.CDLL(so_path)
        if hasattr(lib, "axon_start_nrt_profile"):
            lib.axon_start_nrt_profile.argtypes = [
                ctypes.POINTER(ctypes.c_int64),
                ctypes.c_size_t,
            ]
            lib.axon_start_nrt_profile.restype = ctypes.c_int64
            lib.axon_stop_nrt_profile.argtypes = [ctypes.c_char_p]
            lib.axon_stop_nrt_profile.restype = ctypes.c_int64

            @contextlib.contextmanager
            def _hook(output_dir, device_ids):
                import jax

                jax.devices()
                if device_ids:
                    ids = (ctypes.c_int64 * len(device_ids))(*device_ids)
                    rc = lib.axon_start_nrt_profile(ids, len(device_ids))
                else:
                    rc = lib.axon_start_nrt_profile(None, 0)
                if rc != 0:
                    raise RuntimeError(f"axon_start_nrt_profile rc={rc}")
                try:
                    yield
                finally:
                    n = lib.axon_stop_nrt_profile(str(output_dir).encode())
                    print(f"profile: {n} file(s) written to {output_dir}")

            hook = _hook
    except OSError:
        hook = None

    state = {"hook": hook}
    mod = types.ModuleType("antenv.axon_hooks")
    mod.get_axon_ntff_profile_hook = lambda: state["hook"]
    mod.set_axon_ntff_profile_hook = lambda h: state.__setitem__("hook", h)
    sys.modules["antenv.axon_hooks"] = mod
    try:
        import antenv

        antenv.axon_hooks = mod
    except ImportError:
        pass


def _build_program(w, gate_b, mode="hy", n_tiles=N_TILES):
    import concourse.bacc as bacc
    import concourse.mybir as mybir
    import concourse.tile as tile
    from contextlib import ExitStack

    rows = n_tiles * 128

    F32 = mybir.dt.float32
    F32R = mybir.dt.float32r
    MULT = mybir.AluOpType.mult
    ADD = mybir.AluOpType.add
    BYP = mybir.AluOpType.bypass
    SIG = mybir.ActivationFunctionType.Sigmoid
    IDENT = mybir.ActivationFunctionType.Identity
    COPY = mybir.ActivationFunctionType.Copy

    # weighted-sum chain immediates for the DVE tiles
    a = [float(np.float32(w[i] / w[i + 1])) for i in range(K - 1)]
    w4 = float(np.float32(w[K - 1]))

    nc = bacc.Bacc(
        "TRN2", debug=False, target_bir_lowering=False, num_devices=N_CORES
    )
    cd = nc.dram_tensor("c", [rows, CB], F32, kind="ExternalInput")
    g1d = nc.dram_tensor("g1b", [128, D], F32, kind="ExternalInput")
    g2d = nc.dram_tensor("g2b", [128, D], F32, kind="ExternalInput")
    if mode in ("pe", "hy"):
        dgd = nc.dram_tensor("diag", [128, K * 128], F32, kind="ExternalInput")
    if mode == "hy":
        g2cd = nc.dram_tensor("g2c", [128, D], F32, kind="ExternalInput")
    od = nc.dram_tensor("out", [rows, D], F32, kind="ExternalOutput")

    with tile.TileContext(nc) as tc, ExitStack() as ctx:
        const = ctx.enter_context(tc.tile_pool(name="const", bufs=1))
        cpool = ctx.enter_context(tc.tile_pool(name="cp", bufs=6))
        opool = ctx.enter_context(tc.tile_pool(name="op", bufs=3))
        scrpool = ctx.enter_context(tc.tile_pool(name="scr", bufs=3))
        gpool = ctx.enter_context(tc.tile_pool(name="gp", bufs=4))
        statpool = ctx.enter_context(tc.tile_pool(name="stat", bufs=8))
        if mode in ("pe", "hy"):
            psumpool = ctx.enter_context(
                tc.tile_pool(name="ps", bufs=4, space="PSUM")
            )
        if mode in ("dve", "hy"):
            upool = ctx.enter_context(tc.tile_pool(name="up", bufs=4))
            u4pool = ctx.enter_context(tc.tile_pool(name="u4p", bufs=2))

        # Constants ride the Scalar-engine HWDGE ring (idle at start) so
        # they do not head-block the first combined loads on the Sync ring.
        g1b = const.tile([128, D], F32, tag="g1b")
        nc.scalar.dma_start(g1b[:], g1d.ap())
        g2b = const.tile([128, D], F32, tag="g2b")
        nc.scalar.dma_start(g2b[:], g2d.ap())
        if mode in ("pe", "hy"):
            diag = const.tile([128, K * 128], F32R, tag="diag")
            nc.scalar.dma_start(diag[:], dgd.ap().bitcast(F32R))
        if mode == "hy":
            g2c = const.tile([128, D], F32, tag="g2c")
            nc.scalar.dma_start(g2c[:], g2cd.ap())

        # Supertile schedule: 1-tile supertiles at the start (compute begins
        # after a 1.5 MiB load) and at the end (fine drain granularity);
        # 2-tile supertiles (3 MiB loads) in the steady state.
        if n_tiles >= 8 and (n_tiles - 4) % 2 == 0:
            sched = [1, 1] + [2] * ((n_tiles - 4) // 2) + [1, 1]
        else:
            sched, t = [], n_tiles
            while t > 0:
                s = min(2, t)
                sched.append(s)
                t -= s

        # Tile-granularity DRAM views: [128, n_tiles, F]
        cpv = cd.ap().rearrange("(t p) f -> p t f", p=128)
        opv = od.ap().rearrange("(t p) f -> p t f", p=128)

        # PE consumes the retrieved slices as f32r; load the combined tile
        # as f32r and bitcast back to f32 for the DVE/ScalarE consumers.
        rdt = F32R if mode in ("pe", "hy") else F32

        t0 = 0
        for st, J0 in enumerate(sched):
            c4 = cpool.tile([128, 2 * CB], rdt, tag="c4")
            src = cpv[:, t0 : t0 + J0, :]
            if rdt is F32R:
                src = src.bitcast(F32R)
            # The Sync queue is blocked by ~7 us of runtime preamble at
            # kernel start while the Scalar queue issues from ~0.3 us, so
            # the first supertile loads ride the Scalar ring (behind the
            # tiny constants); the Sync ring takes over the steady state.
            ldeng = nc.scalar if st < 6 else nc.sync
            ldeng.dma_start(
                c4[:, : J0 * CB].rearrange("p (j f) -> p j f", j=J0), src
            )
            o4 = opool.tile([128, 2 * D], F32, tag="o4")

            for j in range(J0):
                def rs(k, f32=False):
                    base = j * CB + k * D
                    ap = c4[:, base : base + D]
                    return ap.bitcast(F32) if (f32 and rdt is F32R) else ap

                qj = c4[:, j * CB + RB : j * CB + CB]
                if rdt is F32R:
                    qj = qj.bitcast(F32)

                # which engine computes `weighted` for this tile (odd tiles
                # on PE so the final tile's drain chain uses the idle PE)
                tile_pe = mode == "pe" or (mode == "hy" and (t0 + j) % 2 == 1)
                if tile_pe:
                    # weighted = sum_k diag(w_k).T @ r_k accumulated in PSUM
                    ps = psumpool.tile([128, D], F32, tag="w")
                    for k in range(K):
                        nc.tensor.matmul(
                            ps[:],
                            diag[:, k * 128 : (k + 1) * 128],
                            rs(k),
                            start=(k == 0),
                            stop=(k == K - 1),
                        )
                    wt_ap = ps[:]
                    gN = 1.0
                    g2x = g2b
                else:
                    # DVE chain: u4 = sum_k (w_k/w4) r_k; w4 folded into
                    # g2c and the gateN scale.
                    u1 = upool.tile([128, D], F32, tag="u")
                    nc.vector.scalar_tensor_tensor(
                        u1[:], rs(0, f32=True), a[0], rs(1, f32=True), MULT, ADD
                    )
                    u2 = upool.tile([128, D], F32, tag="u")
                    nc.vector.scalar_tensor_tensor(
                        u2[:], u1[:], a[1], rs(2, f32=True), MULT, ADD
                    )
                    u3 = upool.tile([128, D], F32, tag="u")
                    nc.vector.scalar_tensor_tensor(
                        u3[:], u2[:], a[2], rs(3, f32=True), MULT, ADD
                    )
                    u4 = u4pool.tile([128, D], F32, tag="u4")
                    nc.vector.scalar_tensor_tensor(
                        u4[:], u3[:], a[3], rs(4, f32=True), MULT, ADD
                    )
                    wt_ap = u4[:]
                    gN = w4
                    g2x = g2c if mode == "hy" else g2b

                # Per-row dots via fused elementwise-mul + accumulate:
                #   s1 = sum(q * g1B),  s2 = sum(weighted * g2B)
                s1 = statpool.tile([128, 1], F32, tag="s1")
                scr1 = scrpool.tile([128, D], F32, tag="scr")
                nc.vector.scalar_tensor_tensor(
                    scr1[:], qj, 0.0, g1b[:], BYP, MULT, accum_out=s1[:]
                )
                s2 = statpool.tile([128, 1], F32, tag="s2")
                scr2 = scrpool.tile([128, D], F32, tag="scr")
                nc.vector.scalar_tensor_tensor(
                    scr2[:], wt_ap, 0.0, g2x[:], BYP, MULT, accum_out=s2[:]
                )
                if gate_b != 0.0:
                    s1b = statpool.tile([128, 1], F32, tag="s1b")
                    nc.gpsimd.tensor_scalar_add(s1b[:], s1[:], gate_b)
                else:
                    s1b = s1

                gate = statpool.tile([128, 1], F32, tag="gate")
                nc.scalar.activation(
                    gate[:], s2[:], SIG, bias=s1b[:], scale=1.0
                )
                # gateN = (1 - gate) * c where the weighted tile holds
                # weighted / c  (c = w4 on DVE tiles, 1 on PE tiles).
                gateN = statpool.tile([128, 1], F32, tag="gateN")
                if gN == 1.0:
                    nc.scalar.activation(
                        gateN[:], gate[:], IDENT, bias=1.0, scale=-1.0
                    )
                else:
                    nc.gpsimd.tensor_scalar(
                        gateN[:], gate[:], -gN, gN, MULT, ADD
                    )

                qg = gpool.tile([128, D], F32, tag="qg")
                nc.scalar.activation(qg[:], qj, COPY, bias=0.0, scale=gate[:])
                wN = gpool.tile([128, D], F32, tag="wN")
                nc.scalar.activation(
                    wN[:], wt_ap, COPY, bias=0.0, scale=gateN[:]
                )
                nc.gpsimd.tensor_add(o4[:, j * D : (j + 1) * D], qg[:], wN[:])

            # Store via the Scalar engine's HWDGE ring so stores do not
            # FIFO-serialize behind the Sync-ring loads.
            nc.scalar.dma_start(
                opv[:, t0 : t0 + J0, :],
                o4[:, : J0 * D].rearrange("p (j f) -> p j f", j=J0),
            )
            t0 += J0

    nc.compile()
    return nc


def kernel(**inputs):
    global LAST_EXEC_NS, LAST_RESULTS

    q = np.ascontiguousarray(np.asarray(inputs["query_embedding"]), dtype=np.float32)
    r = np.ascontiguousarray(
        np.asarray(inputs["retrieved_embeddings"]), dtype=np.float32
    )
    rw = np.asarray(inputs["retrieved_weights"], dtype=np.float64)
    gw = np.asarray(inputs["gate_w"], dtype=np.float64).reshape(-1)
    gb = float(np.asarray(inputs["gate_b"], dtype=np.float64).reshape(-1)[0])

    assert q.shape == (BATCH, D), q.shape
    assert r.shape == (BATCH, K, D), r.shape
    assert rw.shape == (K,), rw.shape
    assert gw.shape == (2 * D,), gw.shape

    # Host: softmax over the 5 slots.
    e = np.exp(rw - rw.max())
    w = e / e.sum()  # float64

    mode = os.environ.get("KERNEL_MODE", "hy")

    g1b = np.ascontiguousarray(
        np.broadcast_to(gw[:D].astype(np.float32), (128, D))
    )
    if mode in ("pe", "hy"):
        g2 = gw[D:]
    else:
        g2 = w[K - 1] * gw[D:]
    g2b = np.ascontiguousarray(np.broadcast_to(g2.astype(np.float32), (128, D)))
    g2c = np.ascontiguousarray(
        np.broadcast_to((w[K - 1] * gw[D:]).astype(np.float32), (128, D))
    )

    key = (mode, tuple(np.float32(w)), gb)
    nc = _PROGRAM_CACHE.get(key)
    if nc is None:
        nc = _build_program(w, gb, mode=mode)
        _PROGRAM_CACHE[key] = nc

    # Combined per-row layout [r0..r4 | q] so the device loads one
    # sequential HBM stream with 12 KiB descriptors.
    comb = np.empty((BATCH, CB), dtype=np.float32)
    comb[:, :RB] = r.reshape(BATCH, RB)
    comb[:, RB:] = q

    in_maps = []
    for c in range(N_CORES):
        lo, hi = c * ROWS, (c + 1) * ROWS
        m = {
            "c": comb[lo:hi],
            "g1b": g1b,
            "g2b": g2b,
        }
        if mode in ("pe", "hy"):
            dg = np.zeros((128, K * 128), dtype=np.float32)
            for k in range(K):
                dg[:, k * 128 : (k + 1) * 128] = np.eye(
                    128, dtype=np.float32
                ) * np.float32(w[k])
            m["diag"] = dg
        if mode == "hy":
            m["g2c"] = g2c
        in_maps.append(m)

    from concourse import bass_utils

    trace = bool(os.environ.get("KERNEL_TRACE"))
    if trace:
        _install_ntff_hook_shim()
        # No S3 in this sandbox; keep profile artifacts local.
        bass_utils.upload_artifacts = lambda tmpdir: tmpdir

    LAST_EXEC_NS = None
    try:
        res = bass_utils.run_bass_kernel_spmd(
            nc, in_maps, core_ids=list(range(N_CORES)), trace=trace
        )
    except Exception:
        if not trace:
            raise
        # Tracing infrastructure failure — rerun without tracing.
        res = bass_utils.run_bass_kernel_spmd(
            nc, in_maps, core_ids=list(range(N_CORES)), trace=False
        )

    LAST_RESULTS = res
    LAST_EXEC_NS = res.exec_time_ns

    out = np.empty((BATCH, D), dtype=np.float32)
    for c in range(N_CORES):
        out[c * ROWS : (c + 1) * ROWS] = res.results[c]["out"]
    return out


# revision 11
# speedup vs baseline: 1.0365x; 1.0365x over previous
"""Trainium2 Bass kernel for the AdvancedFuser problem.

Computes, for each batch row b:
    w        = softmax(retrieved_weights)                       # (5,), host
    weighted = sum_k w[k] * retrieved[b, k, :]                  # (512,)
    gate     = sigmoid(q[b] . g1 + weighted . g2 + gate_b)      # scalar
    out[b]   = gate * q[b] + (1 - gate) * weighted

Sharding: pure data parallel over 8 NeuronCores (8192 rows each). The tiny
params (softmax weights, gate vector) are folded into immediates / small
replicated constant tensors on the host.

The kernel is memory bound: 112 MiB HBM traffic per core (96 in, 16 out)
against the ~358 GB/s per-core HBM share (716 GB/s per stack, 2 cores per
stack) -> ~328 us floor.

Device program (row layout, batch rows on SBUF partitions):
  - The host concatenates retrieved (2560 f32) and q (512 f32) per row into
    one [rows, 3072] tensor so loads are a single sequential HBM stream with
    12 KiB descriptors (vs separate 10 KiB r + 2 KiB q streams).
  - 2-tile supertiles (3 MiB loads), bufs=6 on the load pool: the HWDGE
    FIFO head-of-line wait quantum is halved vs 4-tile supertiles while
    keeping ~18 MiB of prefetch runway, so the SDMA engines stay fed.
    1-tile supertiles at both ends for fast pipeline fill/drain.
  - mode "hy" (default) alternates the weighted-sum engine per 128-row
    tile (even: TensorE 5 accumulating diag(w_k) matmuls in PSUM; odd: DVE
    chain of 4 fused scalar_tensor_tensor with the w_k ratio trick) so
    neither engine alone paces the DMA stream.
  - the two 512-wide per-row dots as fused mul+accumulate on DVE;
    gate = Sigmoid(s2 + s1) on ScalarE; qg = gate*q, wN = gateN*weighted
    via per-partition activation scales on ScalarE; out = qg + wN on GPSIMD.
  - loads ride the Sync HWDGE ring (a pure DMA-issue queue), stores +
    constants the ScalarE HWDGE ring.
"""

import os
import sys

import numpy as np

N_CORES = 8
BATCH = 65536
D = 512
K = 5
RB = K * D  # 2560 floats of retrieved per row
CB = RB + D  # 3072 floats per combined row: [r0..r4 | q]
ROWS = BATCH // N_CORES  # 8192
N_TILES = ROWS // 128  # 64

# Filled by the most recent kernel() call when tracing is enabled.
LAST_EXEC_NS = None
LAST_RESULTS = None

_PROGRAM_CACHE = {}


def _install_ntff_hook_shim():
    """Provide antenv.axon_hooks (missing in this image) so that
    run_bass_kernel_spmd(trace=True) can capture NTFF profiles through the
    axon PJRT .so. Mirrors trn_agent_boot.trn_boot._ntff_profile_via_ctypes."""
    try:
        from antenv.axon_hooks import get_axon_ntff_profile_hook  # noqa: F401

        return
    except ImportError:
        pass
    import contextlib
    import ctypes
    import types

    so_path = "/opt/axon/libaxon_pjrt.so"
    hook = None
    try:
        lib = ctypes.CDLL(so_path)
        if hasattr(lib, "axon_start_nrt_profile"):
            lib.axon_start_nrt_profile.argtypes = [
                ctypes.POINTER(ctypes.c_int64),
                ctypes.c_size_t,
            ]
            lib.axon_start_nrt_profile.restype = ctypes.c_int64
            lib.axon_stop_nrt_profile.argtypes = [ctypes.c_char_p]
            lib.axon_stop_nrt_profile.restype = ctypes.c_int64

            @contextlib.contextmanager
            def _hook(output_dir, device_ids):
                import jax

                jax.devices()
                if device_ids:
                    ids = (ctypes.c_int64 * len(device_ids))(*device_ids)
                    rc = lib.axon_start_nrt_profile(ids, len(device_ids))
                else:
                    rc = lib.axon_start_nrt_profile(None, 0)
                if rc != 0:
                    raise RuntimeError(f"axon_start_nrt_profile rc={rc}")
                try:
                    yield
                finally:
                    n = lib.axon_stop_nrt_profile(str(output_dir).encode())
                    print(f"profile: {n} file(s) written to {output_dir}")

            hook = _hook
    except OSError:
        hook = None

    state = {"hook": hook}
    mod = types.ModuleType("antenv.axon_hooks")
    mod.get_axon_ntff_profile_hook = lambda: state["hook"]
    mod.set_axon_ntff_profile_hook = lambda h: state.__setitem__("hook", h)
    sys.modules["antenv.axon_hooks"] = mod
    try:
        import antenv

        antenv.axon_hooks = mod
    except ImportError:
        pass


def _build_program(w, gate_b, mode="hy", n_tiles=N_TILES):
    import concourse.bacc as bacc
    import concourse.mybir as mybir
    import concourse.tile as tile
    from contextlib import ExitStack

    rows = n_tiles * 128

    F32 = mybir.dt.float32
    F32R = mybir.dt.float32r
    MULT = mybir.AluOpType.mult
    ADD = mybir.AluOpType.add
    BYP = mybir.AluOpType.bypass
    SIG = mybir.ActivationFunctionType.Sigmoid
    IDENT = mybir.ActivationFunctionType.Identity
    COPY = mybir.ActivationFunctionType.Copy

    # weighted-sum chain immediates for the DVE tiles
    a = [float(np.float32(w[i] / w[i + 1])) for i in range(K - 1)]
    w4 = float(np.float32(w[K - 1]))

    nc = bacc.Bacc(
        "TRN2", debug=False, target_bir_lowering=False, num_devices=N_CORES
    )
    cd = nc.dram_tensor("c", [rows, CB], F32, kind="ExternalInput")
    g1d = nc.dram_tensor("g1b", [128, D], F32, kind="ExternalInput")
    g2d = nc.dram_tensor("g2b", [128, D], F32, kind="ExternalInput")
    if mode in ("pe", "hy"):
        dgd = nc.dram_tensor("diag", [128, K * 128], F32, kind="ExternalInput")
    if mode == "hy":
        g2cd = nc.dram_tensor("g2c", [128, D], F32, kind="ExternalInput")
    od = nc.dram_tensor("out", [rows, D], F32, kind="ExternalOutput")

    with tile.TileContext(nc) as tc, ExitStack() as ctx:
        const = ctx.enter_context(tc.tile_pool(name="const", bufs=1))
        cpool = ctx.enter_context(tc.tile_pool(name="cp", bufs=6))
        opool = ctx.enter_context(tc.tile_pool(name="op", bufs=3))
        scrpool = ctx.enter_context(tc.tile_pool(name="scr", bufs=3))
        gpool = ctx.enter_context(tc.tile_pool(name="gp", bufs=4))
        statpool = ctx.enter_context(tc.tile_pool(name="stat", bufs=8))
        if mode in ("pe", "hy"):
            psumpool = ctx.enter_context(
                tc.tile_pool(name="ps", bufs=4, space="PSUM")
            )
        if mode in ("dve", "hy"):
            upool = ctx.enter_context(tc.tile_pool(name="up", bufs=4))
            u4pool = ctx.enter_context(tc.tile_pool(name="u4p", bufs=2))

        # Constants ride the Scalar-engine HWDGE ring (idle at start) so
        # they do not head-block the first combined loads on the Sync ring.
        g1b = const.tile([128, D], F32, tag="g1b")
        nc.scalar.dma_start(g1b[:], g1d.ap())
        g2b = const.tile([128, D], F32, tag="g2b")
        nc.scalar.dma_start(g2b[:], g2d.ap())
        if mode in ("pe", "hy"):
            diag = const.tile([128, K * 128], F32R, tag="diag")
            nc.scalar.dma_start(diag[:], dgd.ap().bitcast(F32R))
        if mode == "hy":
            g2c = const.tile([128, D], F32, tag="g2c")
            nc.scalar.dma_start(g2c[:], g2cd.ap())

        # Supertile schedule: 1-tile supertiles at the start (compute begins
        # after a 1.5 MiB load) and at the end (fine drain granularity);
        # 2-tile supertiles (3 MiB loads) in the steady state.
        if n_tiles >= 8 and (n_tiles - 4) % 2 == 0:
            sched = [1, 1] + [2] * ((n_tiles - 4) // 2) + [1, 1]
        else:
            sched, t = [], n_tiles
            while t > 0:
                s = min(2, t)
                sched.append(s)
                t -= s

        # Tile-granularity DRAM views: [128, n_tiles, F]
        cpv = cd.ap().rearrange("(t p) f -> p t f", p=128)
        opv = od.ap().rearrange("(t p) f -> p t f", p=128)

        # PE consumes the retrieved slices as f32r; load the combined tile
        # as f32r and bitcast back to f32 for the DVE/ScalarE consumers.
        rdt = F32R if mode in ("pe", "hy") else F32

        t0 = 0
        for st, J0 in enumerate(sched):
            c4 = cpool.tile([128, 2 * CB], rdt, tag="c4")
            src = cpv[:, t0 : t0 + J0, :]
            if rdt is F32R:
                src = src.bitcast(F32R)
            nc.sync.dma_start(
                c4[:, : J0 * CB].rearrange("p (j f) -> p j f", j=J0), src
            )
            o4 = opool.tile([128, 2 * D], F32, tag="o4")

            for j in range(J0):
                def rs(k, f32=False):
                    base = j * CB + k * D
                    ap = c4[:, base : base + D]
                    return ap.bitcast(F32) if (f32 and rdt is F32R) else ap

                qj = c4[:, j * CB + RB : j * CB + CB]
                if rdt is F32R:
                    qj = qj.bitcast(F32)

                # which engine computes `weighted` for this tile
                tile_pe = mode == "pe" or (mode == "hy" and (t0 + j) % 2 == 0)
                if tile_pe:
                    # weighted = sum_k diag(w_k).T @ r_k accumulated in PSUM
                    ps = psumpool.tile([128, D], F32, tag="w")
                    for k in range(K):
                        nc.tensor.matmul(
                            ps[:],
                            diag[:, k * 128 : (k + 1) * 128],
                            rs(k),
                            start=(k == 0),
                            stop=(k == K - 1),
                        )
                    wt_ap = ps[:]
                    gN = 1.0
                    g2x = g2b
                else:
                    # DVE chain: u4 = sum_k (w_k/w4) r_k; w4 folded into
                    # g2c and the gateN scale.
                    u1 = upool.tile([128, D], F32, tag="u")
                    nc.vector.scalar_tensor_tensor(
                        u1[:], rs(0, f32=True), a[0], rs(1, f32=True), MULT, ADD
                    )
                    u2 = upool.tile([128, D], F32, tag="u")
                    nc.vector.scalar_tensor_tensor(
                        u2[:], u1[:], a[1], rs(2, f32=True), MULT, ADD
                    )
                    u3 = upool.tile([128, D], F32, tag="u")
                    nc.vector.scalar_tensor_tensor(
                        u3[:], u2[:], a[2], rs(3, f32=True), MULT, ADD
                    )
                    u4 = u4pool.tile([128, D], F32, tag="u4")
                    nc.vector.scalar_tensor_tensor(
                        u4[:], u3[:], a[3], rs(4, f32=True), MULT, ADD
                    )
                    wt_ap = u4[:]
                    gN = w4
                    g2x = g2c if mode == "hy" else g2b

                # Per-row dots via fused elementwise-mul + accumulate:
                #   s1 = sum(q * g1B),  s2 = sum(weighted * g2B)
                s1 = statpool.tile([128, 1], F32, tag="s1")
                scr1 = scrpool.tile([128, D], F32, tag="scr")
                nc.vector.scalar_tensor_tensor(
                    scr1[:], qj, 0.0, g1b[:], BYP, MULT, accum_out=s1[:]
                )
                s2 = statpool.tile([128, 1], F32, tag="s2")
                scr2 = scrpool.tile([128, D], F32, tag="scr")
                nc.vector.scalar_tensor_tensor(
                    scr2[:], wt_ap, 0.0, g2x[:], BYP, MULT, accum_out=s2[:]
                )
                if gate_b != 0.0:
                    s1b = statpool.tile([128, 1], F32, tag="s1b")
                    nc.gpsimd.tensor_scalar_add(s1b[:], s1[:], gate_b)
                else:
                    s1b = s1

                gate = statpool.tile([128, 1], F32, tag="gate")
                nc.scalar.activation(
                    gate[:], s2[:], SIG, bias=s1b[:], scale=1.0
                )
                # gateN = (1 - gate) * c where the weighted tile holds
                # weighted / c  (c = w4 on DVE tiles, 1 on PE tiles).
                gateN = statpool.tile([128, 1], F32, tag="gateN")
                if gN == 1.0:
                    nc.scalar.activation(
                        gateN[:], gate[:], IDENT, bias=1.0, scale=-1.0
                    )
                else:
                    nc.gpsimd.tensor_scalar(
                        gateN[:], gate[:], -gN, gN, MULT, ADD
                    )

                qg = gpool.tile([128, D], F32, tag="qg")
                nc.scalar.activation(qg[:], qj, COPY, bias=0.0, scale=gate[:])
                wN = gpool.tile([128, D], F32, tag="wN")
                nc.scalar.activation(
                    wN[:], wt_ap, COPY, bias=0.0, scale=gateN[:]
                )
                nc.gpsimd.tensor_add(o4[:, j * D : (j + 1) * D], qg[:], wN[:])

            # Store via the Scalar engine's HWDGE ring so stores do not
            # FIFO-serialize behind the Sync-ring loads.
            nc.scalar.dma_start(
                opv[:, t0 : t0 + J0, :],
                o4[:, : J0 * D].rearrange("p (j f) -> p j f", j=J0),
            )
            t0 += J0

    nc.compile()
    return nc


def kernel(**inputs):
    global LAST_EXEC_NS, LAST_RESULTS

    q = np.ascontiguousarray(np.asarray(inputs["query_embedding"]), dtype=np.float32)
    r = np.ascontiguousarray(
        np.asarray(inputs["retrieved_embeddings"]), dtype=np.float32
    )
    rw = np.asarray(inputs["retrieved_weights"], dtype=np.float64)
    gw = np.asarray(inputs["gate_w"], dtype=np.float64).reshape(-1)
    gb = float(np.asarray(inputs["gate_b"], dtype=np.float64).reshape(-1)[0])

    assert q.shape == (BATCH, D), q.shape
    assert r.shape == (BATCH, K, D), r.shape
    assert rw.shape == (K,), rw.shape
    assert gw.shape == (2 * D,), gw.shape

    # Host: softmax over the 5 slots.
    e = np.exp(rw - rw.max())
    w = e / e.sum()  # float64

    mode = os.environ.get("KERNEL_MODE", "hy")

    g1b = np.ascontiguousarray(
        np.broadcast_to(gw[:D].astype(np.float32), (128, D))
    )
    if mode in ("pe", "hy"):
        g2 = gw[D:]
    else:
        g2 = w[K - 1] * gw[D:]
    g2b = np.ascontiguousarray(np.broadcast_to(g2.astype(np.float32), (128, D)))
    g2c = np.ascontiguousarray(
        np.broadcast_to((w[K - 1] * gw[D:]).astype(np.float32), (128, D))
    )

    key = (mode, tuple(np.float32(w)), gb)
    nc = _PROGRAM_CACHE.get(key)
    if nc is None:
        nc = _build_program(w, gb, mode=mode)
        _PROGRAM_CACHE[key] = nc

    # Combined per-row layout [r0..r4 | q] so the device loads one
    # sequential HBM stream with 12 KiB descriptors.
    comb = np.empty((BATCH, CB), dtype=np.float32)
    comb[:, :RB] = r.reshape(BATCH, RB)
    comb[:, RB:] = q

    in_maps = []
    for c in range(N_CORES):
        lo, hi = c * ROWS, (c + 1) * ROWS
        m = {
            "c": comb[lo:hi],
            "g1b": g1b,
            "g2b": g2b,
        }
        if mode in ("pe", "hy"):
            dg = np.zeros((128, K * 128), dtype=np.float32)
            for k in range(K):
                dg[:, k * 128 : (k + 1) * 128] = np.eye(
                    128, dtype=np.float32
                ) * np.float32(w[k])
            m["diag"] = dg
        if mode == "hy":
            m["g2c"] = g2c
        in_maps.append(m)

    from concourse import bass_utils

    trace = bool(os.environ.get("KERNEL_TRACE"))
    if trace:
        _install_ntff_hook_shim()
        # No S3 in this sandbox; keep profile artifacts local.
        bass_utils.upload_artifacts = lambda tmpdir: tmpdir

    LAST_EXEC_NS = None
    try:
        res = bass_utils.run_bass_kernel_spmd(
            nc, in_maps, core_ids=list(range(N_CORES)), trace=trace
        )
    except Exception:
        if not trace:
            raise
        # Tracing infrastructure failure — rerun without tracing.
        res = bass_utils.run_bass_kernel_spmd(
            nc, in_maps, core_ids=list(range(N_CORES)), trace=False
        )

    LAST_RESULTS = res
    LAST_EXEC_NS = res.exec_time_ns

    out = np.empty((BATCH, D), dtype=np.float32)
    for c in range(N_CORES):
        out[c * ROWS : (c + 1) * ROWS] = res.results[c]["out"]
    return out


# revision 13
# speedup vs baseline: 1.0431x; 1.0064x over previous
"""Trainium2 Bass kernel for the AdvancedFuser problem.

Computes, for each batch row b:
    w        = softmax(retrieved_weights)                       # (5,), host
    weighted = sum_k w[k] * retrieved[b, k, :]                  # (512,)
    gate     = sigmoid(q[b] . g1 + weighted . g2 + gate_b)      # scalar
    out[b]   = gate * q[b] + (1 - gate) * weighted

Sharding: pure data parallel over 8 NeuronCores (8192 rows each). The tiny
params (softmax weights, gate vector) are folded into immediates / small
replicated constant tensors on the host.

The kernel is memory bound: 112 MiB HBM traffic per core (96 in, 16 out)
against the ~358 GB/s per-core HBM share (716 GB/s per stack, 2 cores per
stack) -> ~328 us floor.

Device program (row layout, batch rows on SBUF partitions):
  - The host concatenates retrieved (2560 f32) and q (512 f32) per row into
    one [rows, 3072] tensor so loads are a single sequential HBM stream with
    12 KiB descriptors (vs separate 10 KiB r + 2 KiB q streams).
  - 2-tile supertiles (3 MiB loads), bufs=6 on the load pool: the HWDGE
    FIFO head-of-line wait quantum is halved vs 4-tile supertiles while
    keeping ~18 MiB of prefetch runway, so the SDMA engines stay fed.
    1-tile supertiles at both ends for fast pipeline fill/drain.
  - mode "hy" (default) alternates the weighted-sum engine per 128-row
    tile (even: TensorE 5 accumulating diag(w_k) matmuls in PSUM; odd: DVE
    chain of 4 fused scalar_tensor_tensor with the w_k ratio trick) so
    neither engine alone paces the DMA stream.
  - the two 512-wide per-row dots as fused mul+accumulate on DVE;
    gate = Sigmoid(s2 + s1) on ScalarE; qg = gate*q, wN = gateN*weighted
    via per-partition activation scales on ScalarE; out = qg + wN on GPSIMD.
  - loads ride the Sync HWDGE ring (a pure DMA-issue queue), stores +
    constants the ScalarE HWDGE ring.
"""

import os
import sys

import numpy as np

N_CORES = 8
BATCH = 65536
D = 512
K = 5
RB = K * D  # 2560 floats of retrieved per row
CB = RB + D  # 3072 floats per combined row: [r0..r4 | q]
ROWS = BATCH // N_CORES  # 8192
N_TILES = ROWS // 128  # 64

# Filled by the most recent kernel() call when tracing is enabled.
LAST_EXEC_NS = None
LAST_RESULTS = None

_PROGRAM_CACHE = {}


def _install_ntff_hook_shim():
    """Provide antenv.axon_hooks (missing in this image) so that
    run_bass_kernel_spmd(trace=True) can capture NTFF profiles through the
    axon PJRT .so. Mirrors trn_agent_boot.trn_boot._ntff_profile_via_ctypes."""
    try:
        from antenv.axon_hooks import get_axon_ntff_profile_hook  # noqa: F401

        return
    except ImportError:
        pass
    import contextlib
    import ctypes
    import types

    so_path = "/opt/axon/libaxon_pjrt.so"
    hook = None
    try:
        lib = ctypes.CDLL(so_path)
        if hasattr(lib, "axon_start_nrt_profile"):
            lib.axon_start_nrt_profile.argtypes = [
                ctypes.POINTER(ctypes.c_int64),
                ctypes.c_size_t,
            ]
            lib.axon_start_nrt_profile.restype = ctypes.c_int64
            lib.axon_stop_nrt_profile.argtypes = [ctypes.c_char_p]
            lib.axon_stop_nrt_profile.restype = ctypes.c_int64

            @contextlib.contextmanager
            def _hook(output_dir, device_ids):
                import jax

                jax.devices()
                if device_ids:
                    ids = (ctypes.c_int64 * len(device_ids))(*device_ids)
                    rc = lib.axon_start_nrt_profile(ids, len(device_ids))
                else:
                    rc = lib.axon_start_nrt_profile(None, 0)
                if rc != 0:
                    raise RuntimeError(f"axon_start_nrt_profile rc={rc}")
                try:
                    yield
                finally:
                    n = lib.axon_stop_nrt_profile(str(output_dir).encode())
                    print(f"profile: {n} file(s) written to {output_dir}")

            hook = _hook
    except OSError:
        hook = None

    state = {"hook": hook}
    mod = types.ModuleType("antenv.axon_hooks")
    mod.get_axon_ntff_profile_hook = lambda: state["hook"]
    mod.set_axon_ntff_profile_hook = lambda h: state.__setitem__("hook", h)
    sys.modules["antenv.axon_hooks"] = mod
    try:
        import antenv

        antenv.axon_hooks = mod
    except ImportError:
        pass


def _build_program(w, gate_b, mode="hy", n_tiles=N_TILES):
    import concourse.bacc as bacc
    import concourse.mybir as mybir
    import concourse.tile as tile
    from contextlib import ExitStack

    rows = n_tiles * 128

    F32 = mybir.dt.float32
    F32R = mybir.dt.float32r
    MULT = mybir.AluOpType.mult
    ADD = mybir.AluOpType.add
    BYP = mybir.AluOpType.bypass
    SIG = mybir.ActivationFunctionType.Sigmoid
    IDENT = mybir.ActivationFunctionType.Identity
    COPY = mybir.ActivationFunctionType.Copy

    # weighted-sum chain immediates for the DVE tiles
    a = [float(np.float32(w[i] / w[i + 1])) for i in range(K - 1)]
    w4 = float(np.float32(w[K - 1]))

    nc = bacc.Bacc(
        "TRN2", debug=False, target_bir_lowering=False, num_devices=N_CORES
    )
    cd = nc.dram_tensor("c", [rows, CB], F32, kind="ExternalInput")
    g1d = nc.dram_tensor("g1b", [128, D], F32, kind="ExternalInput")
    g2d = nc.dram_tensor("g2b", [128, D], F32, kind="ExternalInput")
    if mode in ("pe", "hy"):
        dgd = nc.dram_tensor("diag", [128, K * 128], F32, kind="ExternalInput")
    if mode == "hy":
        g2cd = nc.dram_tensor("g2c", [128, D], F32, kind="ExternalInput")
    od = nc.dram_tensor("out", [rows, D], F32, kind="ExternalOutput")

    with tile.TileContext(nc) as tc, ExitStack() as ctx:
        const = ctx.enter_context(tc.tile_pool(name="const", bufs=1))
        cpool = ctx.enter_context(tc.tile_pool(name="cp", bufs=6))
        opool = ctx.enter_context(tc.tile_pool(name="op", bufs=3))
        scrpool = ctx.enter_context(tc.tile_pool(name="scr", bufs=3))
        gpool = ctx.enter_context(tc.tile_pool(name="gp", bufs=4))
        statpool = ctx.enter_context(tc.tile_pool(name="stat", bufs=8))
        if mode in ("pe", "hy"):
            psumpool = ctx.enter_context(
                tc.tile_pool(name="ps", bufs=4, space="PSUM")
            )
        if mode in ("dve", "hy"):
            upool = ctx.enter_context(tc.tile_pool(name="up", bufs=4))
            u4pool = ctx.enter_context(tc.tile_pool(name="u4p", bufs=2))

        # Constants ride the Scalar-engine HWDGE ring (idle at start) so
        # they do not head-block the first combined loads on the Sync ring.
        g1b = const.tile([128, D], F32, tag="g1b")
        nc.scalar.dma_start(g1b[:], g1d.ap())
        g2b = const.tile([128, D], F32, tag="g2b")
        nc.scalar.dma_start(g2b[:], g2d.ap())
        if mode in ("pe", "hy"):
            diag = const.tile([128, K * 128], F32R, tag="diag")
            nc.scalar.dma_start(diag[:], dgd.ap().bitcast(F32R))
        if mode == "hy":
            g2c = const.tile([128, D], F32, tag="g2c")
            nc.scalar.dma_start(g2c[:], g2cd.ap())

        # Supertile schedule: 1-tile supertiles at the start (compute begins
        # after a 1.5 MiB load) and at the end (fine drain granularity);
        # 2-tile supertiles (3 MiB loads) in the steady state.
        if n_tiles >= 8 and (n_tiles - 4) % 2 == 0:
            sched = [1, 1] + [2] * ((n_tiles - 4) // 2) + [1, 1]
        else:
            sched, t = [], n_tiles
            while t > 0:
                s = min(2, t)
                sched.append(s)
                t -= s

        # PE consumes the retrieved slices as f32r; load the combined tile
        # as f32r and bitcast back to f32 for the DVE/ScalarE consumers.
        rdt = F32R if mode in ("pe", "hy") else F32

        t0 = 0
        for st, J0 in enumerate(sched):
            # Adjacent-row layout: partition p of this supertile holds DRAM
            # rows t0*128 + J0*p + j (j = 0..J0-1), so each partition's load
            # is ONE contiguous J0*12 KiB chunk (and the store J0*2 KiB) —
            # fewer, larger descriptors and better HBM row locality than the
            # strided (t p) layout. Rows are processed independently and the
            # store below uses the matching view, so semantics are unchanged.
            c4 = cpool.tile([128, 2 * CB], rdt, tag="c4")
            src = cd.ap()[t0 * 128 : (t0 + J0) * 128, :].rearrange(
                "(p j) f -> p (j f)", j=J0
            )
            if rdt is F32R:
                src = src.bitcast(F32R)
            nc.sync.dma_start(c4[:, : J0 * CB], src)
            o4 = opool.tile([128, 2 * D], F32, tag="o4")

            for j in range(J0):
                def rs(k, f32=False):
                    base = j * CB + k * D
                    ap = c4[:, base : base + D]
                    return ap.bitcast(F32) if (f32 and rdt is F32R) else ap

                qj = c4[:, j * CB + RB : j * CB + CB]
                if rdt is F32R:
                    qj = qj.bitcast(F32)

                # which engine computes `weighted` for this tile
                tile_pe = mode == "pe" or (mode == "hy" and (t0 + j) % 2 == 0)
                if tile_pe:
                    # weighted = sum_k diag(w_k).T @ r_k accumulated in PSUM
                    ps = psumpool.tile([128, D], F32, tag="w")
                    for k in range(K):
                        nc.tensor.matmul(
                            ps[:],
                            diag[:, k * 128 : (k + 1) * 128],
                            rs(k),
                            start=(k == 0),
                            stop=(k == K - 1),
                        )
                    wt_ap = ps[:]
                    gN = 1.0
                    g2x = g2b
                else:
                    # DVE chain: u4 = sum_k (w_k/w4) r_k; w4 folded into
                    # g2c and the gateN scale.
                    u1 = upool.tile([128, D], F32, tag="u")
                    nc.vector.scalar_tensor_tensor(
                        u1[:], rs(0, f32=True), a[0], rs(1, f32=True), MULT, ADD
                    )
                    u2 = upool.tile([128, D], F32, tag="u")
                    nc.vector.scalar_tensor_tensor(
                        u2[:], u1[:], a[1], rs(2, f32=True), MULT, ADD
                    )
                    u3 = upool.tile([128, D], F32, tag="u")
                    nc.vector.scalar_tensor_tensor(
                        u3[:], u2[:], a[2], rs(3, f32=True), MULT, ADD
                    )
                    u4 = u4pool.tile([128, D], F32, tag="u4")
                    nc.vector.scalar_tensor_tensor(
                        u4[:], u3[:], a[3], rs(4, f32=True), MULT, ADD
                    )
                    wt_ap = u4[:]
                    gN = w4
                    g2x = g2c if mode == "hy" else g2b

                # Per-row dots via fused elementwise-mul + accumulate:
                #   s1 = sum(q * g1B),  s2 = sum(weighted * g2B)
                s1 = statpool.tile([128, 1], F32, tag="s1")
                scr1 = scrpool.tile([128, D], F32, tag="scr")
                nc.vector.scalar_tensor_tensor(
                    scr1[:], qj, 0.0, g1b[:], BYP, MULT, accum_out=s1[:]
                )
                s2 = statpool.tile([128, 1], F32, tag="s2")
                scr2 = scrpool.tile([128, D], F32, tag="scr")
                nc.vector.scalar_tensor_tensor(
                    scr2[:], wt_ap, 0.0, g2x[:], BYP, MULT, accum_out=s2[:]
                )
                if gate_b != 0.0:
                    s1b = statpool.tile([128, 1], F32, tag="s1b")
                    nc.gpsimd.tensor_scalar_add(s1b[:], s1[:], gate_b)
                else:
                    s1b = s1

                gate = statpool.tile([128, 1], F32, tag="gate")
                nc.scalar.activation(
                    gate[:], s2[:], SIG, bias=s1b[:], scale=1.0
                )
                # gateN = (1 - gate) * c where the weighted tile holds
                # weighted / c  (c = w4 on DVE tiles, 1 on PE tiles).
                gateN = statpool.tile([128, 1], F32, tag="gateN")
                if gN == 1.0:
                    nc.scalar.activation(
                        gateN[:], gate[:], IDENT, bias=1.0, scale=-1.0
                    )
                else:
                    nc.gpsimd.tensor_scalar(
                        gateN[:], gate[:], -gN, gN, MULT, ADD
                    )

                qg = gpool.tile([128, D], F32, tag="qg")
                nc.scalar.activation(qg[:], qj, COPY, bias=0.0, scale=gate[:])
                wN = gpool.tile([128, D], F32, tag="wN")
                nc.scalar.activation(
                    wN[:], wt_ap, COPY, bias=0.0, scale=gateN[:]
                )
                nc.gpsimd.tensor_add(o4[:, j * D : (j + 1) * D], qg[:], wN[:])

            # Store via the Scalar engine's HWDGE ring so stores do not
            # FIFO-serialize behind the Sync-ring loads. Same adjacent-row
            # view as the load, so each output row lands in its true slot.
            nc.scalar.dma_start(
                od.ap()[t0 * 128 : (t0 + J0) * 128, :].rearrange(
                    "(p j) f -> p (j f)", j=J0
                ),
                o4[:, : J0 * D],
            )
            t0 += J0

    nc.compile()
    return nc


def kernel(**inputs):
    global LAST_EXEC_NS, LAST_RESULTS

    q = np.ascontiguousarray(np.asarray(inputs["query_embedding"]), dtype=np.float32)
    r = np.ascontiguousarray(
        np.asarray(inputs["retrieved_embeddings"]), dtype=np.float32
    )
    rw = np.asarray(inputs["retrieved_weights"], dtype=np.float64)
    gw = np.asarray(inputs["gate_w"], dtype=np.float64).reshape(-1)
    gb = float(np.asarray(inputs["gate_b"], dtype=np.float64).reshape(-1)[0])

    assert q.shape == (BATCH, D), q.shape
    assert r.shape == (BATCH, K, D), r.shape
    assert rw.shape == (K,), rw.shape
    assert gw.shape == (2 * D,), gw.shape

    # Host: softmax over the 5 slots.
    e = np.exp(rw - rw.max())
    w = e / e.sum()  # float64

    mode = os.environ.get("KERNEL_MODE", "hy")

    g1b = np.ascontiguousarray(
        np.broadcast_to(gw[:D].astype(np.float32), (128, D))
    )
    if mode in ("pe", "hy"):
        g2 = gw[D:]
    else:
        g2 = w[K - 1] * gw[D:]
    g2b = np.ascontiguousarray(np.broadcast_to(g2.astype(np.float32), (128, D)))
    g2c = np.ascontiguousarray(
        np.broadcast_to((w[K - 1] * gw[D:]).astype(np.float32), (128, D))
    )

    key = (mode, tuple(np.float32(w)), gb)
    nc = _PROGRAM_CACHE.get(key)
    if nc is None:
        nc = _build_program(w, gb, mode=mode)
        _PROGRAM_CACHE[key] = nc

    # Combined per-row layout [r0..r4 | q] so the device loads one
    # sequential HBM stream with 12 KiB descriptors.
    comb = np.empty((BATCH, CB), dtype=np.float32)
    comb[:, :RB] = r.reshape(BATCH, RB)
    comb[:, RB:] = q

    in_maps = []
    for c in range(N_CORES):
        lo, hi = c * ROWS, (c + 1) * ROWS
        m = {
            "c": comb[lo:hi],
            "g1b": g1b,
            "g2b": g2b,
        }
        if mode in ("pe", "hy"):
            dg = np.zeros((128, K * 128), dtype=np.float32)
            for k in range(K):
                dg[:, k * 128 : (k + 1) * 128] = np.eye(
                    128, dtype=np.float32
                ) * np.float32(w[k])
            m["diag"] = dg
        if mode == "hy":
            m["g2c"] = g2c
        in_maps.append(m)

    from concourse import bass_utils

    trace = bool(os.environ.get("KERNEL_TRACE"))
    if trace:
        _install_ntff_hook_shim()
        # No S3 in this sandbox; keep profile artifacts local.
        bass_utils.upload_artifacts = lambda tmpdir: tmpdir

    LAST_EXEC_NS = None
    try:
        res = bass_utils.run_bass_kernel_spmd(
            nc, in_maps, core_ids=list(range(N_CORES)), trace=trace
        )
    except Exception:
        if not trace:
            raise
        # Tracing infrastructure failure — rerun without tracing.
        res = bass_utils.run_bass_kernel_spmd(
            nc, in_maps, core_ids=list(range(N_CORES)), trace=False
        )

    LAST_RESULTS = res
    LAST_EXEC_NS = res.exec_time_ns

    out = np.empty((BATCH, D), dtype=np.float32)
    for c in range(N_CORES):
        out[c * ROWS : (c + 1) * ROWS] = res.results[c]["out"]
    return out


# revision 14
# speedup vs baseline: 1.0453x; 1.0020x over previous
"""Trainium2 Bass kernel for the AdvancedFuser problem.

Computes, for each batch row b:
    w        = softmax(retrieved_weights)                       # (5,), host
    weighted = sum_k w[k] * retrieved[b, k, :]                  # (512,)
    gate     = sigmoid(q[b] . g1 + weighted . g2 + gate_b)      # scalar
    out[b]   = gate * q[b] + (1 - gate) * weighted

Sharding: pure data parallel over 8 NeuronCores (8192 rows each). The tiny
params (softmax weights, gate vector) are folded into immediates / small
replicated constant tensors on the host.

The kernel is memory bound: 112 MiB HBM traffic per core (96 in, 16 out)
against the ~358 GB/s per-core HBM share (716 GB/s per stack, 2 cores per
stack) -> ~328 us floor.

Device program (row layout, batch rows on SBUF partitions):
  - The host concatenates retrieved (2560 f32) and q (512 f32) per row into
    one [rows, 3072] tensor so loads are a single sequential HBM stream with
    12 KiB descriptors (vs separate 10 KiB r + 2 KiB q streams).
  - 2-tile supertiles (3 MiB loads), bufs=6 on the load pool: the HWDGE
    FIFO head-of-line wait quantum is halved vs 4-tile supertiles while
    keeping ~18 MiB of prefetch runway, so the SDMA engines stay fed.
    1-tile supertiles at both ends for fast pipeline fill/drain.
  - mode "hy" (default) alternates the weighted-sum engine per 128-row
    tile (even: TensorE 5 accumulating diag(w_k) matmuls in PSUM; odd: DVE
    chain of 4 fused scalar_tensor_tensor with the w_k ratio trick) so
    neither engine alone paces the DMA stream.
  - the two 512-wide per-row dots as fused mul+accumulate on DVE;
    gate = Sigmoid(s2 + s1) on ScalarE; qg = gate*q, wN = gateN*weighted
    via per-partition activation scales on ScalarE; out = qg + wN on GPSIMD.
  - loads ride the Sync HWDGE ring (a pure DMA-issue queue), stores +
    constants the ScalarE HWDGE ring.
"""

import os
import sys

import numpy as np

N_CORES = 8
BATCH = 65536
D = 512
K = 5
RB = K * D  # 2560 floats of retrieved per row
CB = RB + D  # 3072 floats per combined row: [r0..r4 | q]
ROWS = BATCH // N_CORES  # 8192
N_TILES = ROWS // 128  # 64

# Filled by the most recent kernel() call when tracing is enabled.
LAST_EXEC_NS = None
LAST_RESULTS = None

_PROGRAM_CACHE = {}


def _install_ntff_hook_shim():
    """Provide antenv.axon_hooks (missing in this image) so that
    run_bass_kernel_spmd(trace=True) can capture NTFF profiles through the
    axon PJRT .so. Mirrors trn_agent_boot.trn_boot._ntff_profile_via_ctypes."""
    try:
        from antenv.axon_hooks import get_axon_ntff_profile_hook  # noqa: F401

        return
    except ImportError:
        pass
    import contextlib
    import ctypes
    import types

    so_path = "/opt/axon/libaxon_pjrt.so"
    hook = None
    try:
        lib = ctypes.CDLL(so_path)
        if hasattr(lib, "axon_start_nrt_profile"):
            lib.axon_start_nrt_profile.argtypes = [
                ctypes.POINTER(ctypes.c_int64),
                ctypes.c_size_t,
            ]
            lib.axon_start_nrt_profile.restype = ctypes.c_int64
            lib.axon_stop_nrt_profile.argtypes = [ctypes.c_char_p]
            lib.axon_stop_nrt_profile.restype = ctypes.c_int64

            @contextlib.contextmanager
            def _hook(output_dir, device_ids):
                import jax

                jax.devices()
                if device_ids:
                    ids = (ctypes.c_int64 * len(device_ids))(*device_ids)
                    rc = lib.axon_start_nrt_profile(ids, len(device_ids))
                else:
                    rc = lib.axon_start_nrt_profile(None, 0)
                if rc != 0:
                    raise RuntimeError(f"axon_start_nrt_profile rc={rc}")
                try:
                    yield
                finally:
                    n = lib.axon_stop_nrt_profile(str(output_dir).encode())
                    print(f"profile: {n} file(s) written to {output_dir}")

            hook = _hook
    except OSError:
        hook = None

    state = {"hook": hook}
    mod = types.ModuleType("antenv.axon_hooks")
    mod.get_axon_ntff_profile_hook = lambda: state["hook"]
    mod.set_axon_ntff_profile_hook = lambda h: state.__setitem__("hook", h)
    sys.modules["antenv.axon_hooks"] = mod
    try:
        import antenv

        antenv.axon_hooks = mod
    except ImportError:
        pass


def _build_program(w, gate_b, mode="hy", n_tiles=N_TILES):
    import concourse.bacc as bacc
    import concourse.mybir as mybir
    import concourse.tile as tile
    from contextlib import ExitStack

    rows = n_tiles * 128

    F32 = mybir.dt.float32
    F32R = mybir.dt.float32r
    MULT = mybir.AluOpType.mult
    ADD = mybir.AluOpType.add
    BYP = mybir.AluOpType.bypass
    SIG = mybir.ActivationFunctionType.Sigmoid
    IDENT = mybir.ActivationFunctionType.Identity
    COPY = mybir.ActivationFunctionType.Copy

    # weighted-sum chain immediates for the DVE tiles
    a = [float(np.float32(w[i] / w[i + 1])) for i in range(K - 1)]
    w4 = float(np.float32(w[K - 1]))

    nc = bacc.Bacc(
        "TRN2", debug=False, target_bir_lowering=False, num_devices=N_CORES
    )
    cd = nc.dram_tensor("c", [rows, CB], F32, kind="ExternalInput")
    g1d = nc.dram_tensor("g1b", [128, D], F32, kind="ExternalInput")
    g2d = nc.dram_tensor("g2b", [128, D], F32, kind="ExternalInput")
    if mode in ("pe", "hy"):
        dgd = nc.dram_tensor("diag", [128, K * 128], F32, kind="ExternalInput")
    if mode == "hy":
        g2cd = nc.dram_tensor("g2c", [128, D], F32, kind="ExternalInput")
    od = nc.dram_tensor("out", [rows, D], F32, kind="ExternalOutput")

    with tile.TileContext(nc) as tc, ExitStack() as ctx:
        const = ctx.enter_context(tc.tile_pool(name="const", bufs=1))
        cpool = ctx.enter_context(tc.tile_pool(name="cp", bufs=6))
        opool = ctx.enter_context(tc.tile_pool(name="op", bufs=3))
        scrpool = ctx.enter_context(tc.tile_pool(name="scr", bufs=3))
        gpool = ctx.enter_context(tc.tile_pool(name="gp", bufs=4))
        statpool = ctx.enter_context(tc.tile_pool(name="stat", bufs=8))
        if mode in ("pe", "hy"):
            psumpool = ctx.enter_context(
                tc.tile_pool(name="ps", bufs=4, space="PSUM")
            )
        if mode in ("dve", "hy"):
            upool = ctx.enter_context(tc.tile_pool(name="up", bufs=4))
            u4pool = ctx.enter_context(tc.tile_pool(name="u4p", bufs=2))

        # Constants go FIRST on the Sync ring, ahead of the big loads: the
        # SDMA engines round-robin between queues at packet granularity, so
        # constants placed on the store ring trickle out behind 192 KiB load
        # packets and arrive ~25-35 us late, stalling tile-0 compute (and,
        # via buffer backpressure, the whole load stream). Here they drain
        # in ~1.5 us on otherwise-idle engines before the first load.
        g1b = const.tile([128, D], F32, tag="g1b")
        nc.sync.dma_start(g1b[:], g1d.ap())
        g2b = const.tile([128, D], F32, tag="g2b")
        nc.sync.dma_start(g2b[:], g2d.ap())
        if mode in ("pe", "hy"):
            diag = const.tile([128, K * 128], F32R, tag="diag")
            nc.sync.dma_start(diag[:], dgd.ap().bitcast(F32R))
        if mode == "hy":
            g2c = const.tile([128, D], F32, tag="g2c")
            nc.sync.dma_start(g2c[:], g2cd.ap())

        # Supertile schedule: 1-tile supertiles at the start (compute begins
        # after a 1.5 MiB load) and at the end (fine drain granularity);
        # 2-tile supertiles (3 MiB loads) in the steady state.
        if n_tiles >= 8 and (n_tiles - 4) % 2 == 0:
            sched = [1, 1] + [2] * ((n_tiles - 4) // 2) + [1, 1]
        else:
            sched, t = [], n_tiles
            while t > 0:
                s = min(2, t)
                sched.append(s)
                t -= s

        # PE consumes the retrieved slices as f32r; load the combined tile
        # as f32r and bitcast back to f32 for the DVE/ScalarE consumers.
        rdt = F32R if mode in ("pe", "hy") else F32

        t0 = 0
        for st, J0 in enumerate(sched):
            # Adjacent-row layout: partition p of this supertile holds DRAM
            # rows t0*128 + J0*p + j (j = 0..J0-1), so each partition's load
            # is ONE contiguous J0*12 KiB chunk (and the store J0*2 KiB) —
            # fewer, larger descriptors and better HBM row locality than the
            # strided (t p) layout. Rows are processed independently and the
            # store below uses the matching view, so semantics are unchanged.
            c4 = cpool.tile([128, 2 * CB], rdt, tag="c4")
            src = cd.ap()[t0 * 128 : (t0 + J0) * 128, :].rearrange(
                "(p j) f -> p (j f)", j=J0
            )
            if rdt is F32R:
                src = src.bitcast(F32R)
            nc.sync.dma_start(c4[:, : J0 * CB], src)
            o4 = opool.tile([128, 2 * D], F32, tag="o4")

            for j in range(J0):
                def rs(k, f32=False):
                    base = j * CB + k * D
                    ap = c4[:, base : base + D]
                    return ap.bitcast(F32) if (f32 and rdt is F32R) else ap

                qj = c4[:, j * CB + RB : j * CB + CB]
                if rdt is F32R:
                    qj = qj.bitcast(F32)

                # which engine computes `weighted` for this tile
                tile_pe = mode == "pe" or (mode == "hy" and (t0 + j) % 2 == 0)
                if tile_pe:
                    # weighted = sum_k diag(w_k).T @ r_k accumulated in PSUM
                    ps = psumpool.tile([128, D], F32, tag="w")
                    for k in range(K):
                        nc.tensor.matmul(
                            ps[:],
                            diag[:, k * 128 : (k + 1) * 128],
                            rs(k),
                            start=(k == 0),
                            stop=(k == K - 1),
                        )
                    wt_ap = ps[:]
                    gN = 1.0
                    g2x = g2b
                else:
                    # DVE chain: u4 = sum_k (w_k/w4) r_k; w4 folded into
                    # g2c and the gateN scale.
                    u1 = upool.tile([128, D], F32, tag="u")
                    nc.vector.scalar_tensor_tensor(
                        u1[:], rs(0, f32=True), a[0], rs(1, f32=True), MULT, ADD
                    )
                    u2 = upool.tile([128, D], F32, tag="u")
                    nc.vector.scalar_tensor_tensor(
                        u2[:], u1[:], a[1], rs(2, f32=True), MULT, ADD
                    )
                    u3 = upool.tile([128, D], F32, tag="u")
                    nc.vector.scalar_tensor_tensor(
                        u3[:], u2[:], a[2], rs(3, f32=True), MULT, ADD
                    )
                    u4 = u4pool.tile([128, D], F32, tag="u4")
                    nc.vector.scalar_tensor_tensor(
                        u4[:], u3[:], a[3], rs(4, f32=True), MULT, ADD
                    )
                    wt_ap = u4[:]
                    gN = w4
                    g2x = g2c if mode == "hy" else g2b

                # Per-row dots via fused elementwise-mul + accumulate:
                #   s1 = sum(q * g1B),  s2 = sum(weighted * g2B)
                s1 = statpool.tile([128, 1], F32, tag="s1")
                scr1 = scrpool.tile([128, D], F32, tag="scr")
                nc.vector.scalar_tensor_tensor(
                    scr1[:], qj, 0.0, g1b[:], BYP, MULT, accum_out=s1[:]
                )
                s2 = statpool.tile([128, 1], F32, tag="s2")
                scr2 = scrpool.tile([128, D], F32, tag="scr")
                nc.vector.scalar_tensor_tensor(
                    scr2[:], wt_ap, 0.0, g2x[:], BYP, MULT, accum_out=s2[:]
                )
                if gate_b != 0.0:
                    s1b = statpool.tile([128, 1], F32, tag="s1b")
                    nc.gpsimd.tensor_scalar_add(s1b[:], s1[:], gate_b)
                else:
                    s1b = s1

                gate = statpool.tile([128, 1], F32, tag="gate")
                nc.scalar.activation(
                    gate[:], s2[:], SIG, bias=s1b[:], scale=1.0
                )
                # gateN = (1 - gate) * c where the weighted tile holds
                # weighted / c  (c = w4 on DVE tiles, 1 on PE tiles).
                gateN = statpool.tile([128, 1], F32, tag="gateN")
                if gN == 1.0:
                    nc.scalar.activation(
                        gateN[:], gate[:], IDENT, bias=1.0, scale=-1.0
                    )
                else:
                    nc.gpsimd.tensor_scalar(
                        gateN[:], gate[:], -gN, gN, MULT, ADD
                    )

                qg = gpool.tile([128, D], F32, tag="qg")
                nc.scalar.activation(qg[:], qj, COPY, bias=0.0, scale=gate[:])
                wN = gpool.tile([128, D], F32, tag="wN")
                nc.scalar.activation(
                    wN[:], wt_ap, COPY, bias=0.0, scale=gateN[:]
                )
                nc.gpsimd.tensor_add(o4[:, j * D : (j + 1) * D], qg[:], wN[:])

            # Store via the Scalar engine's HWDGE ring so stores do not
            # FIFO-serialize behind the Sync-ring loads. Same adjacent-row
            # view as the load, so each output row lands in its true slot.
            nc.scalar.dma_start(
                od.ap()[t0 * 128 : (t0 + J0) * 128, :].rearrange(
                    "(p j) f -> p (j f)", j=J0
                ),
                o4[:, : J0 * D],
            )
            t0 += J0

    nc.compile()
    return nc


def kernel(**inputs):
    global LAST_EXEC_NS, LAST_RESULTS

    q = np.ascontiguousarray(np.asarray(inputs["query_embedding"]), dtype=np.float32)
    r = np.ascontiguousarray(
        np.asarray(inputs["retrieved_embeddings"]), dtype=np.float32
    )
    rw = np.asarray(inputs["retrieved_weights"], dtype=np.float64)
    gw = np.asarray(inputs["gate_w"], dtype=np.float64).reshape(-1)
    gb = float(np.asarray(inputs["gate_b"], dtype=np.float64).reshape(-1)[0])

    assert q.shape == (BATCH, D), q.shape
    assert r.shape == (BATCH, K, D), r.shape
    assert rw.shape == (K,), rw.shape
    assert gw.shape == (2 * D,), gw.shape

    # Host: softmax over the 5 slots.
    e = np.exp(rw - rw.max())
    w = e / e.sum()  # float64

    mode = os.environ.get("KERNEL_MODE", "hy")

    g1b = np.ascontiguousarray(
        np.broadcast_to(gw[:D].astype(np.float32), (128, D))
    )
    if mode in ("pe", "hy"):
        g2 = gw[D:]
    else:
        g2 = w[K - 1] * gw[D:]
    g2b = np.ascontiguousarray(np.broadcast_to(g2.astype(np.float32), (128, D)))
    g2c = np.ascontiguousarray(
        np.broadcast_to((w[K - 1] * gw[D:]).astype(np.float32), (128, D))
    )

    key = (mode, tuple(np.float32(w)), gb)
    nc = _PROGRAM_CACHE.get(key)
    if nc is None:
        nc = _build_program(w, gb, mode=mode)
        _PROGRAM_CACHE[key] = nc

    # Combined per-row layout [r0..r4 | q] so the device loads one
    # sequential HBM stream with 12 KiB descriptors.
    comb = np.empty((BATCH, CB), dtype=np.float32)
    comb[:, :RB] = r.reshape(BATCH, RB)
    comb[:, RB:] = q

    in_maps = []
    for c in range(N_CORES):
        lo, hi = c * ROWS, (c + 1) * ROWS
        m = {
            "c": comb[lo:hi],
            "g1b": g1b,
            "g2b": g2b,
        }
        if mode in ("pe", "hy"):
            dg = np.zeros((128, K * 128), dtype=np.float32)
            for k in range(K):
                dg[:, k * 128 : (k + 1) * 128] = np.eye(
                    128, dtype=np.float32
                ) * np.float32(w[k])
            m["diag"] = dg
        if mode == "hy":
            m["g2c"] = g2c
        in_maps.append(m)

    from concourse import bass_utils

    trace = bool(os.environ.get("KERNEL_TRACE"))
    if trace:
        _install_ntff_hook_shim()
        # No S3 in this sandbox; keep profile artifacts local.
        bass_utils.upload_artifacts = lambda tmpdir: tmpdir

    LAST_EXEC_NS = None
    try:
        res = bass_utils.run_bass_kernel_spmd(
            nc, in_maps, core_ids=list(range(N_CORES)), trace=trace
        )
    except Exception:
        if not trace:
            raise
        # Tracing infrastructure failure — rerun without tracing.
        res = bass_utils.run_bass_kernel_spmd(
            nc, in_maps, core_ids=list(range(N_CORES)), trace=False
        )

    LAST_RESULTS = res
    LAST_EXEC_NS = res.exec_time_ns

    out = np.empty((BATCH, D), dtype=np.float32)
    for c in range(N_CORES):
        out[c * ROWS : (c + 1) * ROWS] = res.results[c]["out"]
    return out
